# revision 30
# baseline (speedup 1.0000x reference)
"""Additive attention (nn_AdditiveAttention) distributed Bass kernel for 8 TRN2 cores.

Reference math (per batch b):
    k = key @ Wk                  (NK, H)
    q = query @ Wq                (NQ, H)
    scores[ki, qi] = sum_h wv[h] * tanh(k[ki, h] + q[qi, h])
    masked = where(qi < valid_lens[b], scores, -1e6)
    attn = softmax(masked, axis=qi)
    out = attn @ value            (NK, DV)

Key facts used:
  * Masked q-columns produce attn == 0 exactly (exp(-1e6 - m) underflows to 0 in
    fp32), so columns qi >= valid_len contribute nothing to output or denominator.
    The kernel therefore only processes qi < Ts where Ts = per-slot trip count =
    max valid_len over the cores' slot-s batches, rounded up to the chunk size.
  * softmax without max-subtraction is safe: |scores| <= sum|wv| ~ 10.

Sharding: data-parallel over batch. Each core processes 2 batches ("slots");
slot 0 gets the 8 largest valid_lens, slot 1 the 8 smallest, so the SPMD-static
trip counts (T0, T1) stay near sum(vl)/8 of real work.

Per-q work (the dominant cost), chunked by CH=32 q-columns:
  DVE:     sum[:, j, :] = kT(bf16) + qT[:, q]      (tensor_scalar add)
  ScalarE: tanh in-place over the whole (128, CH*256) chunk (one big ACTIVATE)
  TensorE: per q: scores column = feat_blk^T(bf16) @ wv into PSUM (k-part, q-free)
"""

import numpy as np

import concourse.bass as bass
import concourse.bacc as bacc
import concourse.tile as tile
from concourse import mybir
from concourse.bass_utils import run_bass_kernel_spmd

B = 16
NK = 256
NQ = 256
DK = 256
DV = 256
H = 128
P = 128
NCORES = 8
SLOTS = 2
CH = 32  # q-columns per tanh chunk
MASK_VAL = -1000000.0

F32 = mybir.dt.float32
BF16 = mybir.dt.bfloat16
I32 = mybir.dt.int32
TANH = mybir.ActivationFunctionType.Tanh
EXP = mybir.ActivationFunctionType.Exp
ADD = mybir.AluOpType.add

_CACHE = {}


def _qblocks(t):
    """Split t query-rows into PE-contraction blocks of <=128 rows."""
    blocks = []
    off = 0
    while off < t:
        n = min(P, t - off)
        blocks.append((off, n))
        off += n
    return blocks


def _build(trips):
    nc = bacc.Bacc("TRN2", target_bir_lowering=False, debug=False, num_devices=NCORES)

    key_d = nc.dram_tensor("keyx", [SLOTS, NK, DK], F32, kind="ExternalInput")
    query_d = nc.dram_tensor("queryx", [SLOTS, NQ, DK], F32, kind="ExternalInput")
    value_d = nc.dram_tensor("valuex", [SLOTS, NQ, DV], F32, kind="ExternalInput")
    vlf_d = nc.dram_tensor("vlf", [SLOTS], F32, kind="ExternalInput")
    wk_d = nc.dram_tensor("Wk", [DK, H], F32, kind="ExternalInput")
    wq_d = nc.dram_tensor("Wq", [DK, H], F32, kind="ExternalInput")
    wv_d = nc.dram_tensor("wv", [H, 1], F32, kind="ExternalInput")
    id_d = nc.dram_tensor("ident", [P, P], F32, kind="ExternalInput")
    out_d = nc.dram_tensor("out", [SLOTS, NK, DV], F32, kind="ExternalOutput")

    NKB = NK // P
    DKB = DK // P
    QMAX = max(trips)

    with tile.TileContext(nc) as tc:
        with (
            tc.tile_pool(name="const", bufs=1) as const,
            tc.tile_pool(name="big", bufs=1) as big,
            tc.tile_pool(name="work", bufs=2) as work,
            tc.tile_pool(name="chunk", bufs=3) as chunk_pool,
            tc.tile_pool(name="ps_sc", bufs=2, space="PSUM") as ps_sc,
            tc.tile_pool(name="ps_tmp", bufs=2, space="PSUM") as ps_tmp,
        ):
            # ---- tiles ----
            wkf = const.tile([P, DKB, H], F32)
            wqf = const.tile([P, DKB, H], F32)
            wk_sb = const.tile([P, DKB, H], BF16)
            wq_sb = const.tile([P, DKB, H], BF16)
            wvf = const.tile([P, 1], F32)
            wv_sb = const.tile([P, 1], BF16)
            idf = const.tile([P, P], F32)
            id_sb = const.tile([P, P], BF16)
            vl_sb = const.tile([P, SLOTS], F32)
            iota_sb = const.tile([P, NQ], I32)
            iotaf_sb = const.tile([P, NQ], F32)
            mask_sb = const.tile([P, SLOTS, NQ], F32)

            kT_sb = big.tile([P, SLOTS, NK], BF16)
            qT_sb = big.tile([P, SLOTS, NQ], F32)
            val_sb = big.tile([P, SLOTS, NQ // P, DV], BF16)
            attn_sb = big.tile([P, SLOTS, NKB, QMAX], BF16)
            attnT_sb = big.tile([P, SLOTS, (QMAX + P - 1) // P, NK], BF16)
            rec_sb = big.tile([P, SLOTS, NKB], F32)

            natf = {}
            for s in range(SLOTS):
                for name in ("k", "q"):
                    natf[name, s] = work.tile(
                        [P, NKB, DK], F32, name=f"natf_{name}{s}", tag=f"natf_{name}{s}"
                    )

            # ---- DMAs: critical loads first, split across sync + gpsimd queues ----
            for nb in range(NKB):
                nc.sync.dma_start(
                    out=natf["k", 0][:, nb, :], in_=key_d[0, nb * P : (nb + 1) * P, :]
                )
            nc.gpsimd.dma_start(out=idf, in_=id_d[:, :])
            nc.gpsimd.dma_start(out=wvf, in_=wv_d[:, :])
            for nb in range(NKB):
                nc.sync.dma_start(
                    out=natf["q", 0][:, nb, :], in_=query_d[0, nb * P : (nb + 1) * P, :]
                )
            for i in range(DKB):
                nc.gpsimd.dma_start(out=wkf[:, i, :], in_=wk_d[i * P : (i + 1) * P, :])
            for i in range(DKB):
                nc.gpsimd.dma_start(out=wqf[:, i, :], in_=wq_d[i * P : (i + 1) * P, :])
            for nb in range(NKB):
                nc.gpsimd.dma_start(
                    out=natf["k", 1][:, nb, :], in_=key_d[1, nb * P : (nb + 1) * P, :]
                )
                nc.gpsimd.dma_start(
                    out=natf["q", 1][:, nb, :], in_=query_d[1, nb * P : (nb + 1) * P, :]
                )
            vlf_ap = vlf_d.ap()
            vlf_bcast = bass.AP(
                tensor=vlf_ap.tensor, offset=vlf_ap.offset, ap=[[0, P]] + list(vlf_ap.ap)
            )
            nc.sync.dma_start(out=vl_sb, in_=vlf_bcast)



            # ---- phase A: per slot, kT/qT = (x @ W)^T ----
            # nb=0 block first for both tensors so kT + qT[:, :128] are ready early
            def phase_a(s):
                xTs = {}
                for name in ("k", "q"):
                    xTs[name] = work.tile(
                        [P, DKB, NK], BF16, name=f"xT_{name}{s}", tag=f"xT_{name}{s}"
                    )
                if s == 0:
                    nc.vector.tensor_copy(wk_sb[:, :, :], wkf[:, :, :])
                    nc.vector.tensor_copy(wq_sb[:, :, :], wqf[:, :, :])

                def do_block(name, nb):
                    nat, xT = natf[name, s], xTs[name]
                    for db in range(DKB):
                        tp = ps_tmp.tile(
                            [P, P], F32, name=f"tp_{name}{s}{nb}{db}", tag="tp"
                        )
                        nc.tensor.transpose(
                            tp, nat[:, nb, db * P : (db + 1) * P], idf
                        )
                        if s == 0 and nb == 0:
                            # ACT is idle during the head; keep DVE free
                            nc.scalar.copy(xT[:, db, nb * P : (nb + 1) * P], tp)
                        else:
                            nc.vector.tensor_copy(xT[:, db, nb * P : (nb + 1) * P], tp)

                def do_prj(name, w_sb, dstT, nb, out_dt_split):
                    xT = xTs[name]
                    prj = ps_tmp.tile(
                        [P, P], F32, name=f"prj_{name}{s}{nb}", tag="prj"
                    )
                    for db in range(DKB):
                        nc.tensor.matmul(
                            prj,
                            w_sb[:, db, :],
                            xT[:, db, nb * P : (nb + 1) * P],
                            start=(db == 0),
                            stop=(db == DKB - 1),
                        )
                    lo = nb * P
                    if s != 0:
                        nc.vector.tensor_copy(dstT[:, s, lo : lo + P], prj)
                    elif out_dt_split:
                        nc.scalar.copy(dstT[:, s, lo : lo + CH], prj[:, :CH])
                        nc.scalar.copy(dstT[:, s, lo + CH : lo + P], prj[:, CH:])
                    else:
                        nc.scalar.copy(dstT[:, s, lo : lo + P], prj)

                steps = [
                    lambda: do_block("k", 0),
                    lambda: do_block("q", 0),
                    lambda: do_prj("k", wk_sb, kT_sb, 0, False),
                    lambda: do_prj("q", wq_sb, qT_sb, 0, s == 0),
                    lambda: do_block("k", 1),
                    lambda: do_block("q", 1),
                    lambda: do_prj("k", wk_sb, kT_sb, 1, False),
                    lambda: do_prj("q", wq_sb, qT_sb, 1, False),
                ]
                if s == 0:
                    for st in steps:
                        st()
                    return []
                return steps

            phase_a(0)

            def _masks():
                nc.vector.tensor_copy(id_sb, idf)
                nc.gpsimd.iota(iota_sb, pattern=[[1, NQ]], base=0, channel_multiplier=0)
                nc.vector.tensor_copy(iotaf_sb, iota_sb)
                for s in range(SLOTS):
                    nc.vector.tensor_scalar(
                        out=mask_sb[:, s, :],
                        in0=iotaf_sb,
                        scalar1=vl_sb[:, s : s + 1],
                        scalar2=MASK_VAL,
                        op0=mybir.AluOpType.is_ge,
                        op1=mybir.AluOpType.mult,
                    )

            def _value(s, qb):
                def go():
                    vf = work.tile([P, DV], F32, name=f"vf{s}{qb}", tag="vf")
                    nc.gpsimd.dma_start(
                        out=vf, in_=value_d[s, qb * P : (qb + 1) * P, :]
                    )
                    nc.vector.tensor_copy(val_sb[:, s, qb, :], vf)
                return go

            # ---- phase B+C: chunk streams with slot interleave ----
            def chunk_sizes(s, T, start=0):
                sizes = []
                if s == 0:
                    for r in (8, 16):
                        if start + sum(sizes) + r <= T:
                            sizes.append(r)
                done = start + sum(sizes)
                while done < T:
                    g = min(CH, T - done)
                    sizes.append(g)
                    done += g
                if s == SLOTS - 1 and sizes[-1] > 16:
                    g = sizes.pop()
                    sizes += [g - g // 2, g // 2]
                return sizes

            sc_tiles = {}
            av_tiles = {}

            NBIAS = 8  # leading slot-0 columns done as bias-fused tanh on ACT

            def emit_bias_cols(s, n):
                feat_t = chunk_pool.tile([P, CH, NK], BF16, name="feat_t", tag="feat")
                for j in range(n):
                    nc.scalar.activation(
                        out=feat_t[:, j, :],
                        in_=kT_sb[:, s, :],
                        func=TANH,
                        bias=qT_sb[:, s, j : j + 1],
                        scale=1.0,
                    )
                    for kb in range(NKB):
                        nc.tensor.matmul(
                            sc_tiles[s][kb][:, j : j + 1],
                            feat_t[:, j, kb * P : (kb + 1) * P],
                            wv_sb,
                            start=True,
                            stop=True,
                        )

            def emit_chunk(s, c0, g):
                sum_t = chunk_pool.tile([P, CH, NK], BF16, name="sum_t", tag="sum")
                feat_t = chunk_pool.tile([P, CH, NK], BF16, name="feat_t", tag="feat")
                for j in range(g):
                    nc.vector.tensor_scalar_add(
                        out=sum_t[:, j, :],
                        in0=kT_sb[:, s, :],
                        scalar1=qT_sb[:, s, c0 + j : c0 + j + 1],
                    )
                nc.scalar.activation(
                    out=feat_t[:, :g, :], in_=sum_t[:, :g, :], func=TANH
                )
                for j in range(g):
                    for kb in range(NKB):
                        nc.tensor.matmul(
                            sc_tiles[s][kb][:, c0 + j : c0 + j + 1],
                            feat_t[:, j, kb * P : (kb + 1) * P],
                            wv_sb,
                            start=True,
                            stop=True,
                        )

            part_dens = {}

            def _epi_range(s, lo, hi, first, last):
                sc_ps = sc_tiles[s]
                den_p = work.tile(
                    [P, NKB], F32, name=f"den{s}p{int(first)}", tag=f"den{int(first)}"
                )
                for kb in range(NKB):
                    nc.vector.tensor_tensor(
                        out=sc_ps[kb][:, lo:hi],
                        in0=sc_ps[kb][:, lo:hi],
                        in1=mask_sb[:, s, lo:hi],
                        op=ADD,
                    )
                    nc.scalar.activation(
                        out=attn_sb[:, s, kb, lo:hi],
                        in_=sc_ps[kb][:, lo:hi],
                        func=EXP,
                    )
                    nc.vector.reduce_sum(
                        out=den_p[:, kb : kb + 1],
                        in_=attn_sb[:, s, kb, lo:hi],
                        axis=mybir.AxisListType.X,
                    )
                    n = hi - lo
                    qb = lo // P
                    tp2 = ps_tmp.tile(
                        [P, P], BF16, name=f"tp2_{s}{kb}{lo}", tag="tp"
                    )
                    nc.tensor.transpose(tp2[:n, :], attn_sb[:, s, kb, lo:hi], id_sb)
                    nc.vector.tensor_copy(
                        attnT_sb[lo:hi, s, qb, kb * P : (kb + 1) * P], tp2[:n, :]
                    )
                    nc.tensor.matmul(
                        av_tiles[s][kb],
                        attnT_sb[lo:hi, s, qb, kb * P : (kb + 1) * P],
                        val_sb[:, s, qb, :][lo - qb * P : hi - qb * P, :],
                        start=first,
                        stop=last,
                    )
                return den_p

            def _epi_finish(s, dens):
                den = dens[0]
                for den_p in dens[1:]:
                    nc.vector.tensor_tensor(out=den, in0=den, in1=den_p, op=ADD)
                nc.vector.reciprocal(rec_sb[:, s, :], den)
                for kb in range(NKB):
                    o_sb = work.tile([P, DV], F32, name=f"o{s}{kb}", tag="o")
                    nc.vector.tensor_scalar(
                        out=o_sb,
                        in0=av_tiles[s][kb],
                        scalar1=rec_sb[:, s, kb : kb + 1],
                        scalar2=None,
                        op0=mybir.AluOpType.mult,
                    )
                    nc.sync.dma_start(out=out_d[s, kb * P : (kb + 1) * P, :], in_=o_sb)

            def _epi_full(s, T):
                den = work.tile([P, NKB], F32, name=f"den{s}", tag="den0")
                sc_ps = sc_tiles[s]
                for kb in range(NKB):
                    nc.vector.tensor_tensor(
                        out=sc_ps[kb][:, :T],
                        in0=sc_ps[kb][:, :T],
                        in1=mask_sb[:, s, :T],
                        op=ADD,
                    )
                    nc.scalar.activation(
                        out=attn_sb[:, s, kb, :T],
                        in_=sc_ps[kb][:, :T],
                        func=EXP,
                    )
                    nc.vector.reduce_sum(
                        out=den[:, kb : kb + 1],
                        in_=attn_sb[:, s, kb, :T],
                        axis=mybir.AxisListType.X,
                    )
                nc.vector.reciprocal(rec_sb[:, s, :], den)

                qblocks = _qblocks(T)
                for kb in range(NKB):
                    for qb, (off, n) in enumerate(qblocks):
                        tp2 = ps_tmp.tile([P, P], BF16, name=f"tp2_{s}{kb}{qb}", tag="tp")
                        nc.tensor.transpose(
                            tp2[:n, :], attn_sb[:, s, kb, off : off + n], id_sb
                        )
                        nc.vector.tensor_copy(
                            attnT_sb[:n, s, qb, kb * P : (kb + 1) * P], tp2[:n, :]
                        )

                for kb in range(NKB):
                    av = av_tiles[s][kb]
                    for qb, (off, n) in enumerate(qblocks):
                        nc.tensor.matmul(
                            av,
                            attnT_sb[:n, s, qb, kb * P : (kb + 1) * P],
                            val_sb[:, s, off // P, :][0:n, :],
                            start=(qb == 0),
                            stop=(qb == len(qblocks) - 1),
                        )
                    o_sb = work.tile([P, DV], F32, name=f"o{s}{kb}", tag="o")
                    nc.vector.tensor_scalar(
                        out=o_sb,
                        in0=av,
                        scalar1=rec_sb[:, s, kb : kb + 1],
                        scalar2=None,
                        op0=mybir.AluOpType.mult,
                    )
                    nc.sync.dma_start(out=out_d[s, kb * P : (kb + 1) * P, :], in_=o_sb)

            plans = []
            for s in range(SLOTS):
                T = trips[s]
                sc_tiles[s] = [
                    ps_sc.tile([P, QMAX], F32, name=f"sc{s}{kb}", tag=f"sc{kb}")
                    for kb in range(NKB)
                ]
                av_tiles[s] = [
                    ps_tmp.tile([P, DV], F32, name=f"av{s}{kb}", tag="prj")
                    for kb in range(NKB)
                ]
                start = 8 if (s == 0 and T > 16) else 0
                sizes = chunk_sizes(s, T, start=start)
                offs = [start + sum(sizes[:i]) for i in range(len(sizes))]
                plans.append(list(zip(offs, sizes)))

            nc.vector.tensor_copy(wv_sb, wvf)
            if trips[0] > 16:
                emit_bias_cols(0, NBIAS)
            deferred = phase_a(1) + [_masks] + [
                _value(s2, qb) for s2 in range(SLOTS) for qb in range(NQ // P)
            ]
            for idx, (c0, g) in enumerate(plans[0]):
                emit_chunk(0, c0, g)
                if idx >= 2:
                    for _ in range(2):
                        if deferred:
                            deferred.pop(0)()
            while deferred:
                deferred.pop(0)()
            emit_chunk(1, *plans[1][0])
            _epi_full(0, trips[0])
            T1 = trips[1]
            if 64 < T1 <= P and len(plans[1]) >= 3:
                done = plans[1][0][1]
                i = 1
                while done < 64:
                    emit_chunk(1, *plans[1][i])
                    done += plans[1][i][1]
                    i += 1
                den_a = _epi_range(1, 0, 64, True, False)
                for c0, g in plans[1][i:]:
                    emit_chunk(1, c0, g)
                den_b = _epi_range(1, 64, T1, False, True)
                _epi_finish(1, [den_a, den_b])
            else:
                for c0, g in plans[1][1:]:
                    emit_chunk(1, c0, g)
                emit_epilogue(1, T1)

    nc.compile()
    return nc


def kernel(key, query, value, valid_lens, Wk, Wq, wv, _trace=False):
    key = np.ascontiguousarray(np.asarray(key, dtype=np.float32))
    query = np.ascontiguousarray(np.asarray(query, dtype=np.float32))
    value = np.ascontiguousarray(np.asarray(value, dtype=np.float32))
    valid_lens = np.asarray(valid_lens)
    Wk = np.ascontiguousarray(np.asarray(Wk, dtype=np.float32))
    Wq = np.ascontiguousarray(np.asarray(Wq, dtype=np.float32))
    wv = np.ascontiguousarray(np.asarray(wv, dtype=np.float32)).reshape(H, 1)
    ident = np.eye(P, dtype=np.float32)

    vl = np.clip(valid_lens.astype(np.int64), 1, NQ)
    order = np.argsort(-vl, kind="stable")  # descending
    slot0 = order[:NCORES]
    slot1 = order[NCORES:][::-1]
    assign = list(zip(slot0.tolist(), slot1.tolist()))

    def _trip(batches):
        m = int(vl[batches].max())
        return min(NQ, -(-m // 8) * 8)

    trips = (_trip(slot0), _trip(slot1))

    if trips not in _CACHE:
        _CACHE[trips] = _build(trips)
    nc = _CACHE[trips]

    in_maps = []
    for b0, b1 in assign:
        ids = [b0, b1]
        in_maps.append(
            {
                "keyx": key[ids],
                "queryx": query[ids],
                "valuex": value[ids],
                "vlf": valid_lens[ids].astype(np.float32),
                "Wk": Wk,
                "Wq": Wq,
                "wv": wv,
                "ident": ident,
            }
        )

    res = run_bass_kernel_spmd(nc, in_maps, core_ids=list(range(NCORES)), trace=_trace)
    kernel.last_results = res

    out = np.empty((B, NK, DV), dtype=np.float32)
    for c, (b0, b1) in enumerate(assign):
        shard = res.results[c]["out"]
        out[b0] = shard[0]
        out[b1] = shard[1]
    return out
